# revision 18
# baseline (speedup 1.0000x reference)
"""Trainium2 Bass kernel for CRFExtensionModule (conv3x3 backbone + 5 mean-field
CRF iterations with separable Gaussian blur).

Strategy (per NeuronCore, 2 images of the 16-image batch):
  - C=2 softmax collapses to a sigmoid of d = logit1 - logit0, and
    blur(q0) = blur(ones) - blur(q1), so the whole CRF loop is a single-plane
    recurrence:  d' = (du - ob) + 2*blur(sigmoid(d)),  ob = blur(ones).
  - conv3x3 computes only the planes u1 and du = u1 - u0 via banded matmuls on
    the TensorEngine (band stationary, x moving), with tiny K=6 "fix" matmuls
    for the 2 rows per 128-row tile boundary that the aligned K window misses.
  - blur = two *transposing* banded matmul passes on the TensorEngine:
    pass1: UT[w,h'] = sum_h s[h,w] A[h,h']   (lhsT = s block, rhs = A band)
    pass2: V[h,w'] = sum_w UT[w,h] A[w,w']   (lhsT = UT block, rhs = A band)
    Output lands back in [h, w] layout - no explicit transposes anywhere.
  - ob = blur(ones) = outer(v, v) is rank-1; M = ob - du is formed once per
    image and injected into PSUM with K=128 identity matmuls.
  - Final iteration: out1 = u1 + blur(s), out0 = (u1 + M) - blur(s) are both
    assembled fully in PSUM (plain/negated A + identity injects) and DMA'd
    directly PSUM -> DRAM.

kernel(**inputs) takes the FULL inputs and returns the FULL output.
"""

import os
import sys
from contextlib import ExitStack

sys.path.insert(0, "/opt/trn_rl_repo")

import numpy as np
import ml_dtypes

import concourse.bass as bass
import concourse.bacc as bacc
import concourse.tile as tile
import concourse.mybir as mybir
from concourse.bass_utils import run_bass_kernel_spmd

F32 = mybir.dt.float32
BF16 = mybir.dt.bfloat16

# dtype knobs (bf16 halves TensorEngine streaming / weight-load time)
CONV_BF16 = os.environ.get("KERNEL_CONV_BF16", "0") == "1"
BLUR_BF16 = os.environ.get("KERNEL_BLUR_BF16", "1") == "1"

N_CORES = 8
IMGS_PER_CORE = 2
H = W = 512
NT = 4  # 128-row tiles per image plane
N_ITER = 5
FILT = 11


def _gauss_k():
    d = np.arange(FILT, dtype=np.float32) - np.float32((FILT - 1) / 2.0)
    k = np.exp(-(d ** 2) / np.float32(2.0)).astype(np.float32)
    return (k / k.sum()).astype(np.float32)


def _make_A(scale):
    """A[h, h'] = k[h-h'+5] for |h-h'| <= 5 (zero-padded 'SAME' 1D blur)."""
    k = (_gauss_k() * np.float32(scale)).astype(np.float32)
    A = np.zeros((H, H), np.float32)
    hp = np.arange(H)
    for j in range(FILT):
        h = hp + (j - 5)
        m = (h >= 0) & (h < H)
        A[h[m], hp[m]] = k[j]
    return A


def _win(t):
    """h' window that rows [128t, 128t+128) of A touch."""
    return max(0, 128 * t - 5), min(H, 128 * t + 133)


def _np_dt(dt):
    return ml_dtypes.bfloat16 if dt == BF16 else np.float32


# ---------------------------------------------------------------------------
# kernel body (traced once; shared SPMD program for all 8 cores)
# ---------------------------------------------------------------------------


def _build(nc, tc, conv_dt, blur_dt):
    x_d = nc.dram_tensor("x", [IMGS_PER_CORE, 3, H, W], F32, kind="ExternalInput").ap()
    y_d = nc.dram_tensor("y", [IMGS_PER_CORE, 2, H, W], F32, kind="ExternalOutput").ap()
    bands_d = nc.dram_tensor("bands", [128, 18, 128], conv_dt, kind="ExternalInput").ap()
    wf_d = nc.dram_tensor("wf", [6, 6, 128], conv_dt, kind="ExternalInput").ap()
    As_d = nc.dram_tensor("A_s", [128, NT, H], blur_dt, kind="ExternalInput").ap()
    Ap_d = nc.dram_tensor("A_p", [128, NT, H], blur_dt, kind="ExternalInput").ap()
    negI_d = nc.dram_tensor("negI", [128, 128], F32, kind="ExternalInput").ap()
    I_d = nc.dram_tensor("ident", [128, 128], F32, kind="ExternalInput").ap()
    ob_d = nc.dram_tensor("ob", [128, NT, W], F32, kind="ExternalInput").ap()
    biases_d = nc.dram_tensor("biases", [128, 2], F32, kind="ExternalInput").ap()

    ALU = mybir.AluOpType
    AF = mybir.ActivationFunctionType

    with ExitStack() as ctx:
        cpool = ctx.enter_context(tc.tile_pool(name="consts", bufs=1))
        spool = ctx.enter_context(tc.tile_pool(name="sbuf", bufs=2))
        xpool = ctx.enter_context(tc.tile_pool(name="xin", bufs=1))
        ppool = ctx.enter_context(
            tc.tile_pool(name="psum", bufs=8, space=bass.MemorySpace.PSUM))

        def psum():
            return ppool.tile([128, 512], F32, tag="ps", name="ps")

        def band_mms(P, lhsT_of, A_t, last_extra=0):
            """Banded blur matmuls into one PSUM bank; each window split into
            an accumulate-only overlap strip and a fresh strip (HW has
            per-element has_written, CoreSim requires purity)."""
            written = 0
            for t in range(NT):
                lo, hi = _win(t)
                if lo < written:
                    nc.tensor.matmul(
                        P[:, lo:written], lhsT_of(t), A_t(t, lo, written),
                        start=False, stop=False, skip_group_check=True)
                flo = max(lo, written)
                is_last = (t == NT - 1) and last_extra == 0
                nc.tensor.matmul(
                    P[:, flo:hi], lhsT_of(t), A_t(t, flo, hi),
                    start=(t == 0), stop=is_last, skip_group_check=True)
                written = hi

        # --- constants into SBUF (once; all dtype-matched HWDGE loads) ---
        bands = cpool.tile([128, 18, 128], conv_dt, tag="bands")
        nc.scalar.dma_start(bands[:], bands_d)
        wf = cpool.tile([6, 6, 128], conv_dt, tag="wf")
        nc.scalar.dma_start(wf[:], wf_d)
        A_s = cpool.tile([128, NT, H], blur_dt, tag="A_s")
        A_p = cpool.tile([128, NT, H], blur_dt, tag="A_p")
        nc.scalar.dma_start(A_s[:], As_d)
        nc.scalar.dma_start(A_p[:], Ap_d)
        negI = cpool.tile([128, 128], F32, tag="negI")
        nc.scalar.dma_start(negI[:], negI_d)
        ident = cpool.tile([128, 128], F32, tag="ident")
        nc.scalar.dma_start(ident[:], I_d)
        ob = cpool.tile([128, NT, W], F32, tag="ob")
        nc.scalar.dma_start(ob[:], ob_d)
        biases = cpool.tile([128, 2], F32, tag="biases")
        nc.scalar.dma_start(biases[:], biases_d)

        for im in range(IMGS_PER_CORE):
            ring = nc.sync

            # ---- x load: f32 over HWDGE, cast to conv_dt on GpSimd ----
            xtf = xpool.tile([128, 3, NT, W + 2], F32, tag="xtf")
            nc.gpsimd.memset(xtf[:, :, :, 0:1], 0.0)
            nc.gpsimd.memset(xtf[:, :, :, W + 1:W + 2], 0.0)
            src = x_d[im].rearrange("c (b p) w -> p c b w", p=128)
            ring.dma_start(xtf[:, :, :, 1:W + 1], src)
            if conv_dt == F32:
                xt = xtf
            else:
                xt = xpool.tile([128, 3, NT, W + 2], conv_dt, tag=f"xt{im}")
                nc.gpsimd.tensor_copy(xt[:], xtf[:])

            # boundary-fix rows: xb[b][3r + c, 1+j];
            #   r=0 (parts 0-2) -> x row 128b-1, r=1 (parts 3-5) -> 128b+128
            xbt = []
            for b in range(NT):
                xbf = xpool.tile([6, W + 2], F32, tag=f"xbf{im}{b}")
                nc.gpsimd.memset(xbf[:], 0.0)
                if b > 0:
                    ring.dma_start(xbf[0:3, 1:W + 1], x_d[im, :, 128 * b - 1, :])
                if b < NT - 1:
                    ring.dma_start(xbf[3:6, 1:W + 1], x_d[im, :, 128 * b + 128, :])
                if conv_dt == F32:
                    xbt.append(xbf)
                else:
                    xb = xpool.tile([6, W + 2], conv_dt, tag=f"xb{im}{b}")
                    nc.gpsimd.tensor_copy(xb[:], xbf[:])
                    xbt.append(xb)

            # ---- conv: u1 / du planes (+ bias), one PSUM bank per (set, b) ----
            u1 = [spool.tile([128, W], F32, tag=f"u1_{b}", name=f"u1_{b}") for b in range(NT)]
            du = [spool.tile([128, W], F32, tag=f"du_{b}", name=f"du_{b}") for b in range(NT)]
            M = [spool.tile([128, W], F32, tag=f"M_{b}", name=f"M_{b}") for b in range(NT)]
            P2 = [spool.tile([128, W], F32, tag=f"P2_{b}", name=f"P2_{b}") for b in range(NT)]
            for b in range(NT):
                for set_i in range(2):
                    P = psum()
                    n_mm = 0
                    for c in range(3):
                        for kx in range(3):
                            nc.tensor.matmul(
                                P[:], bands[:, set_i * 9 + c * 3 + kx, :],
                                xt[:, c, b, kx:kx + 512],
                                start=(n_mm == 0), stop=False,
                                skip_group_check=True)
                            n_mm += 1
                    for kx in range(3):
                        nc.tensor.matmul(
                            P[:], wf[:, set_i * 3 + kx, :],
                            xbt[b][:, kx:kx + 512],
                            start=False, stop=(kx == 2), skip_group_check=True)
                    if set_i == 0:
                        # u1 = conv1 + b1   (ACT)
                        nc.scalar.activation(
                            u1[b][:], P[:], AF.Identity,
                            bias=biases[:, 0:1], scale=1.0)
                    else:
                        # du = (conv1-conv0) + (b1-b0)   (DVE)
                        nc.vector.tensor_scalar(
                            du[b][:], P[:], biases[:, 1:2], None, ALU.add)
                # M = ob - du ; P2 = 2*u1 + M (for final out0)
                nc.vector.tensor_sub(M[b][:], ob[:, b, :], du[b][:])
                nc.vector.scalar_tensor_tensor(
                    P2[b][:], u1[b][:], 2.0, M[b][:], ALU.mult, ALU.add)

            # ---- CRF iterations ----
            prev_dp = None  # 4 single-bank psum tiles holding d' chunks
            for it in range(N_ITER):
                s_sb = [spool.tile([128, W], blur_dt, tag=f"s_{t}", name=f"s_{t}")
                        for t in range(NT)]
                for t in range(NT):
                    sig_in = du[t][:] if it == 0 else prev_dp[t][:]
                    nc.scalar.activation(s_sb[t][:], sig_in, AF.Sigmoid)

                A_iter = A_s if it < N_ITER - 1 else A_p
                # pass 1: UT[w, h'] chunks (transposing banded blur along h)
                ut = [spool.tile([128, H], blur_dt, tag=f"ut_{s}", name=f"ut_{s}")
                      for s in range(NT)]
                for s in range(NT):
                    UTP = psum()
                    band_mms(UTP,
                             lambda t, s=s: s_sb[t][:, 128 * s:128 * s + 128],
                             lambda t, a, b2: A_iter[:, t, a:b2])
                    nc.vector.tensor_copy(ut[s][:], UTP[:])

                # pass 2: V[h, w'] chunks back in row layout (+ injects)
                if it < N_ITER - 1:
                    new_dp = []
                    for tp in range(NT):
                        DP = psum()
                        band_mms(DP,
                                 lambda s, tp=tp: ut[s][:, 128 * tp:128 * tp + 128],
                                 lambda s, a, b2: A_iter[:, s, a:b2],
                                 last_extra=1)
                        # d' = 2*blur(s) - M
                        nc.tensor.matmul(
                            DP[:], negI[:], M[tp][:],
                            start=False, stop=True, skip_group_check=True)
                        new_dp.append(DP)
                    prev_dp = new_dp
                else:
                    # final: DP1 = blur(s) + u1 = out1 ; out0 = P2 - DP1
                    for tp in range(NT):
                        DP1 = psum()
                        band_mms(DP1,
                                 lambda s, tp=tp: ut[s][:, 128 * tp:128 * tp + 128],
                                 lambda s, a, b2: A_p[:, s, a:b2],
                                 last_extra=1)
                        nc.tensor.matmul(
                            DP1[:], ident[:], u1[tp][:],
                            start=False, stop=True, skip_group_check=True)
                        o1 = spool.tile([128, W], F32, tag=f"o1_{tp}", name=f"o1_{tp}")
                        o0 = spool.tile([128, W], F32, tag=f"o0_{tp}", name=f"o0_{tp}")
                        nc.scalar.copy(o1[:], DP1[:])
                        nc.vector.scalar_tensor_tensor(
                            o0[:], DP1[:], -1.0, P2[tp][:], ALU.mult, ALU.add)
                        ring2 = nc.sync if tp % 2 == 0 else nc.scalar
                        ring2.dma_start(y_d[im, 1, 128 * tp:128 * tp + 128, :], o1[:])
                        ring2.dma_start(y_d[im, 0, 128 * tp:128 * tp + 128, :], o0[:])


_CACHE = {}


def _get_compiled(conv_bf16=None, blur_bf16=None):
    conv_bf16 = CONV_BF16 if conv_bf16 is None else conv_bf16
    blur_bf16 = BLUR_BF16 if blur_bf16 is None else blur_bf16
    key = (conv_bf16, blur_bf16)
    if key in _CACHE:
        return _CACHE[key]
    conv_dt = BF16 if conv_bf16 else F32
    blur_dt = BF16 if blur_bf16 else F32
    nc = bacc.Bacc(
        "TRN2",
        target_bir_lowering=False,
        debug=False,
        enable_asserts=False,
        num_devices=N_CORES,
    )
    with tile.TileContext(nc) as tc:
        _build(nc, tc, conv_dt, blur_dt)
    nc.compile()
    _CACHE[key] = nc
    return nc


def host_constants(conv_w, conv_b, conv_bf16=None, blur_bf16=None):
    """All weight-derived device constants, as numpy arrays."""
    conv_bf16 = CONV_BF16 if conv_bf16 is None else conv_bf16
    blur_bf16 = BLUR_BF16 if blur_bf16 is None else blur_bf16
    w = np.asarray(conv_w, np.float32)
    b = np.asarray(conv_b, np.float32)
    sets = [w[1] + 0.0, w[1] - w[0]]  # u1-plane, du-plane (3,3,3) each

    bands = np.zeros((128, 18, 128), np.float32)
    r = np.arange(128)
    for set_i, ws in enumerate(sets):
        for c in range(3):
            for kx in range(3):
                Band = np.zeros((128, 128), np.float32)
                for ky in range(3):
                    m = r - (ky - 1)
                    ok = (m >= 0) & (m < 128)
                    Band[r[ok], m[ok]] = ws[c, ky, kx]
                bands[:, set_i * 9 + c * 3 + kx, :] = Band

    wf = np.zeros((6, 6, 128), np.float32)
    for set_i, ws in enumerate(sets):
        for kx in range(3):
            WF = np.zeros((6, 128), np.float32)
            for c in range(3):
                WF[0 + c, 0] = ws[c, 0, kx]      # r=0 rows: x row 128b-1, ky=0
                WF[3 + c, 127] = ws[c, 2, kx]    # r=1 rows: x row 128b+128, ky=2
            wf[:, set_i * 3 + kx, :] = WF

    def tile4(A):
        return np.ascontiguousarray(A.reshape(NT, 128, H).transpose(1, 0, 2))

    A_s = tile4(_make_A(np.sqrt(np.float32(2.0))))
    A_p = tile4(_make_A(1.0))

    k = _gauss_k()
    v = np.convolve(np.ones(H, np.float32), k, mode="same").astype(np.float32)
    ob_full = np.outer(v, v).astype(np.float32)  # blur(ones), rank-1
    ob = np.ascontiguousarray(ob_full.reshape(NT, 128, W).transpose(1, 0, 2))

    ident = np.eye(128, dtype=np.float32)
    cdt = ml_dtypes.bfloat16 if conv_bf16 else np.float32
    bdt = ml_dtypes.bfloat16 if blur_bf16 else np.float32
    b1, db = np.float32(b[1]), np.float32(b[1] - b[0])
    return {
        "bands": bands.astype(cdt),
        "wf": wf.astype(cdt),
        "A_s": A_s.astype(bdt),
        "A_p": A_p.astype(bdt),
        "negI": (-ident).astype(np.float32),
        "ident": ident,
        "ob": ob,
        "biases": np.tile(np.array([[b1, db]], np.float32), (128, 1)),
    }


def _install_ntff_hook_shim():
    """This container's antenv lacks axon_hooks; recreate the NTFF profile
    hook via ctypes into libaxon_pjrt.so (same ABI trn_boot.py uses).
    Only invoked for traced (profiling) runs."""
    import types
    import ctypes
    import contextlib

    try:
        from antenv.axon_hooks import get_axon_ntff_profile_hook  # noqa: F401
        return
    except ImportError:
        pass

    hook = None
    so_path = "/opt/axon/libaxon_pjrt.so"
    if os.path.exists(so_path):
        lib = ctypes.CDLL(so_path)
        if hasattr(lib, "axon_start_nrt_profile"):
            lib.axon_start_nrt_profile.argtypes = [
                ctypes.POINTER(ctypes.c_int64), ctypes.c_size_t,
            ]
            lib.axon_start_nrt_profile.restype = ctypes.c_int64
            lib.axon_stop_nrt_profile.argtypes = [ctypes.c_char_p]
            lib.axon_stop_nrt_profile.restype = ctypes.c_int64

            @contextlib.contextmanager
            def _hook(output_dir, device_ids):
                import jax

                jax.devices()
                if device_ids:
                    ids = (ctypes.c_int64 * len(device_ids))(*device_ids)
                    rc = lib.axon_start_nrt_profile(ids, len(device_ids))
                else:
                    rc = lib.axon_start_nrt_profile(None, 0)
                if rc != 0:
                    raise RuntimeError(f"axon_start_nrt_profile rc={rc}")
                try:
                    yield
                finally:
                    n = lib.axon_stop_nrt_profile(str(output_dir).encode())
                    print(f"profile: {n} file(s) written to {output_dir}", file=sys.stderr)

            hook = _hook

    import antenv

    mod = types.ModuleType("antenv.axon_hooks")
    mod.get_axon_ntff_profile_hook = lambda: hook
    mod.set_axon_ntff_profile_hook = lambda h: None
    sys.modules["antenv.axon_hooks"] = mod
    antenv.axon_hooks = mod


def kernel(x, conv_w, conv_b, _trace=False, _return_results=False):
    if _trace:
        _install_ntff_hook_shim()
    x = np.ascontiguousarray(np.asarray(x, np.float32))
    consts = host_constants(conv_w, conv_b)

    nc = _get_compiled()
    in_maps = []
    for core in range(N_CORES):
        m = {"x": np.ascontiguousarray(x[IMGS_PER_CORE * core:IMGS_PER_CORE * (core + 1)])}
        m.update(consts)
        in_maps.append(m)

    res = run_bass_kernel_spmd(nc, in_maps, core_ids=list(range(N_CORES)), trace=_trace)
    out = np.concatenate([res.results[c]["y"] for c in range(N_CORES)], axis=0).astype(np.float32)
    if _return_results:
        return out, res
    return out


if __name__ == "__main__":
    rng = np.random.default_rng(0)
    x = rng.standard_normal((16, 3, H, W), dtype=np.float32)
    w = (rng.standard_normal((2, 3, 3, 3)) * 0.1).astype(np.float32)
    b = np.zeros(2, np.float32)
    y = kernel(x=x, conv_w=w, conv_b=b)
    print("out", y.shape, y.dtype)


# revision 21
# speedup vs baseline: 1.9677x; 1.9677x over previous
"""Trainium2 Bass kernel for CRFExtensionModule (conv3x3 backbone + 5 mean-field
CRF iterations with separable Gaussian blur).

Strategy (per NeuronCore, 2 images of the 16-image batch):
  - C=2 softmax collapses to a sigmoid of d = logit1 - logit0, and
    blur(q0) = blur(ones) - blur(q1), so the whole CRF loop is a single-plane
    recurrence:  d' = (du - ob) + 2*blur(sigmoid(d)),  ob = blur(ones).
  - conv3x3 computes only the planes u1 and du = u1 - u0 via banded matmuls on
    the TensorEngine (band stationary, x moving), with tiny K=6 "fix" matmuls
    for the 2 rows per 128-row tile boundary that the aligned K window misses.
  - blur = two *transposing* banded matmul passes on the TensorEngine:
    pass1: UT[w,h'] = sum_h s[h,w] A[h,h']   (lhsT = s block, rhs = A band)
    pass2: V[h,w'] = sum_w UT[w,h] A[w,w']   (lhsT = UT block, rhs = A band)
    Output lands back in [h, w] layout - no explicit transposes anywhere.
  - ob = blur(ones) = outer(v, v) is rank-1; M = ob - du is formed once per
    image and injected into PSUM with K=128 identity matmuls.
  - Final iteration: out1 = u1 + blur(s), out0 = (u1 + M) - blur(s) are both
    assembled fully in PSUM (plain/negated A + identity injects) and DMA'd
    directly PSUM -> DRAM.

kernel(**inputs) takes the FULL inputs and returns the FULL output.
"""

import os
import sys
from contextlib import ExitStack

sys.path.insert(0, "/opt/trn_rl_repo")

import numpy as np
import ml_dtypes

import concourse.bass as bass
import concourse.bacc as bacc
import concourse.tile as tile
import concourse.mybir as mybir
from concourse.bass_utils import run_bass_kernel_spmd

F32 = mybir.dt.float32
BF16 = mybir.dt.bfloat16
FP16 = mybir.dt.float16

# dtype knobs (bf16 halves TensorEngine streaming / weight-load time)
CONV_BF16 = os.environ.get("KERNEL_CONV_BF16", "1") == "1"
BLUR_BF16 = os.environ.get("KERNEL_BLUR_BF16", "1") == "1"

N_CORES = 8
IMGS_PER_CORE = 2
H = W = 512
NT = 4  # 128-row tiles per image plane
N_ITER = 5
FILT = 11


def _gauss_k():
    d = np.arange(FILT, dtype=np.float32) - np.float32((FILT - 1) / 2.0)
    k = np.exp(-(d ** 2) / np.float32(2.0)).astype(np.float32)
    return (k / k.sum()).astype(np.float32)


def _make_A(scale):
    """A[h, h'] = k[h-h'+5] for |h-h'| <= 5 (zero-padded 'SAME' 1D blur)."""
    k = (_gauss_k() * np.float32(scale)).astype(np.float32)
    A = np.zeros((H, H), np.float32)
    hp = np.arange(H)
    for j in range(FILT):
        h = hp + (j - 5)
        m = (h >= 0) & (h < H)
        A[h[m], hp[m]] = k[j]
    return A


def _win(t):
    """h' window that rows [128t, 128t+128) of A touch."""
    return max(0, 128 * t - 5), min(H, 128 * t + 133)


def _np_dt(dt):
    return ml_dtypes.bfloat16 if dt == BF16 else np.float32


# ---------------------------------------------------------------------------
# kernel body (traced once; shared SPMD program for all 8 cores)
# ---------------------------------------------------------------------------


def _build(nc, tc, conv_dt, blur_dt):
    x_d = nc.dram_tensor("x", [IMGS_PER_CORE, 3, H, W], F32, kind="ExternalInput").ap()
    y_d = nc.dram_tensor("y", [IMGS_PER_CORE, 2, H, W], F32, kind="ExternalOutput").ap()
    bands_d = nc.dram_tensor("bands", [128, 18, 128], conv_dt, kind="ExternalInput").ap()
    wf_d = nc.dram_tensor("wf", [35, 6, 128], conv_dt, kind="ExternalInput").ap()
    As_d = nc.dram_tensor("A_s", [128, NT, H], blur_dt, kind="ExternalInput").ap()
    Ap_d = nc.dram_tensor("A_p", [128, NT, H], blur_dt, kind="ExternalInput").ap()
    negI_d = nc.dram_tensor("negI", [128, 128], FP16, kind="ExternalInput").ap()
    ob_d = nc.dram_tensor("ob", [128, NT, W], F32, kind="ExternalInput").ap()
    biases_d = nc.dram_tensor("biases", [128, 2], F32, kind="ExternalInput").ap()

    ALU = mybir.AluOpType
    AF = mybir.ActivationFunctionType

    with ExitStack() as ctx:
        cpool = ctx.enter_context(tc.tile_pool(name="consts", bufs=1))
        spool = ctx.enter_context(tc.tile_pool(name="sbuf", bufs=2))
        xpool = ctx.enter_context(tc.tile_pool(name="xin", bufs=1))
        ppool = ctx.enter_context(
            tc.tile_pool(name="psum", bufs=8, space=bass.MemorySpace.PSUM))

        def psum():
            return ppool.tile([128, 512], F32, tag="ps", name="ps")

        def band_mms(P, lhsT_of, A_t, last_extra=0):
            """Banded blur matmuls into one PSUM bank; each window split into
            an accumulate-only overlap strip and a fresh strip (HW has
            per-element has_written, CoreSim requires purity)."""
            written = 0
            for t in range(NT):
                lo, hi = _win(t)
                if lo < written:
                    nc.tensor.matmul(
                        P[:, lo:written], lhsT_of(t), A_t(t, lo, written),
                        start=False, stop=False, skip_group_check=True)
                flo = max(lo, written)
                is_last = (t == NT - 1) and last_extra == 0
                nc.tensor.matmul(
                    P[:, flo:hi], lhsT_of(t), A_t(t, flo, hi),
                    start=(t == 0), stop=is_last, skip_group_check=True)
                written = hi

        # --- constants into SBUF (once; all dtype-matched HWDGE loads) ---
        bands = cpool.tile([128, 18, 128], conv_dt, tag="bands")
        nc.scalar.dma_start(bands[:], bands_d)
        wf = cpool.tile([35, 6, 128], conv_dt, tag="wf")
        nc.scalar.dma_start(wf[:], wf_d)
        A_s = cpool.tile([128, NT, H], blur_dt, tag="A_s")
        A_p = cpool.tile([128, NT, H], blur_dt, tag="A_p")
        nc.scalar.dma_start(A_s[:], As_d)
        nc.scalar.dma_start(A_p[:], Ap_d)
        negI = cpool.tile([128, 128], FP16, tag="negI")
        nc.scalar.dma_start(negI[:], negI_d)
        ob = cpool.tile([128, NT, W], F32, tag="ob")
        nc.scalar.dma_start(ob[:], ob_d)
        biases = cpool.tile([128, 2], F32, tag="biases")
        nc.scalar.dma_start(biases[:], biases_d)

        for im in range(IMGS_PER_CORE):
            # ---- boundary-fix rows first (SWDGE cast ring warms up early):
            #   xb[b][3r + c, j]; r=0 (parts 0-2) -> x row 128b-1, r=1 -> 128b+128
            xbt = []
            for b in range(NT):
                # r=0 rows at partitions 0-2, r=1 rows at 32-34 (SWDGE dst
                # partition starts must be 32-aligned)
                xb = xpool.tile([35, W], conv_dt, tag=f"xb{im}{b}", name=f"xb{im}{b}")
                nc.gpsimd.memset(xb[:], 0.0)
                if b > 0:
                    nc.gpsimd.dma_start(xb[0:3, :], x_d[im, :, 128 * b - 1, :])
                if b < NT - 1:
                    nc.gpsimd.dma_start(xb[32:35, :], x_d[im, :, 128 * b + 128, :])
                xbt.append(xb)

            # ---- x planes: img0 via HWDGE f32 + per-channel GpSimd cast
            #      (fast start);  img1 via one SWDGE cast-DMA (overlapped) ----
            if conv_dt == F32:
                xt = xpool.tile([128, 3, NT, W], F32, tag=f"xt{im}", name=f"xt{im}")
                nc.sync.dma_start(
                    xt[:], x_d[im].rearrange("c (b p) w -> p c b w", p=128))
            elif im == 0:
                xtf = xpool.tile([128, 3, NT, W], F32, tag="xtf", name="xtf")
                xt = xpool.tile([128, 3, NT, W], conv_dt, tag=f"xt{im}", name=f"xt{im}")
                for c in range(3):
                    nc.sync.dma_start(
                        xtf[:, c, :, :],
                        x_d[im, c].rearrange("(b p) w -> p b w", p=128))
                    nc.gpsimd.tensor_copy(xt[:, c, :, :], xtf[:, c, :, :])
            else:
                xt = xpool.tile([128, 3, NT, W], conv_dt, tag=f"xt{im}", name=f"xt{im}")
                nc.gpsimd.dma_start(
                    xt[:], x_d[im].rearrange("c (b p) w -> p c b w", p=128))

            # ---- conv: u1 / du planes (+ bias), one PSUM bank per (set, b) ----
            u1 = [spool.tile([128, W], F32, tag=f"u1_{b}", name=f"u1_{b}") for b in range(NT)]
            du = [spool.tile([128, W], F32, tag=f"du_{b}", name=f"du_{b}") for b in range(NT)]
            M = [spool.tile([128, W], FP16, tag=f"M_{b}", name=f"M_{b}") for b in range(NT)]
            P2 = [spool.tile([128, W], F32, tag=f"P2_{b}", name=f"P2_{b}") for b in range(NT)]
            for b in range(NT):
                for set_i in range(2):
                    P = psum()
                    n_mm = 0
                    for c in range(3):
                        for kx in (1, 0, 2):  # center first: full-bank start=True
                            # kx=0 reads x[.., j-1]: src cols [0,511) -> out [1,512)
                            # kx=2 reads x[.., j+1]: src cols [1,512) -> out [0,511)
                            sl, ol = (0, 1) if kx == 0 else (1, 0) if kx == 2 else (0, 0)
                            n = W - (1 if kx != 1 else 0)
                            nc.tensor.matmul(
                                P[:, ol:ol + n],
                                bands[:, set_i * 9 + c * 3 + kx, :],
                                xt[:, c, b, sl:sl + n],
                                start=(n_mm == 0), stop=False,
                                skip_group_check=True)
                            n_mm += 1
                    for kx in (1, 0, 2):
                        sl, ol = (0, 1) if kx == 0 else (1, 0) if kx == 2 else (0, 0)
                        n = W - (1 if kx != 1 else 0)
                        nc.tensor.matmul(
                            P[:, ol:ol + n], wf[:, set_i * 3 + kx, :],
                            xbt[b][:, sl:sl + n],
                            start=False, stop=(kx == 2), skip_group_check=True)
                    if set_i == 0:
                        # u1 = conv1 + b1   (ACT)
                        nc.scalar.activation(
                            u1[b][:], P[:], AF.Identity,
                            bias=biases[:, 0:1], scale=1.0)
                    else:
                        # du = (conv1-conv0) + (b1-b0)   (DVE)
                        nc.vector.tensor_scalar(
                            du[b][:], P[:], biases[:, 1:2], None, ALU.add)
                # M = ob - du (fp16: inject operand) ; P2 = u1 + M
                nc.vector.tensor_sub(M[b][:], ob[:, b, :], du[b][:])
                nc.vector.tensor_add(P2[b][:], u1[b][:], M[b][:])

            # ---- CRF iterations ----
            prev_dp = None  # 4 single-bank psum tiles holding d' chunks
            for it in range(N_ITER):
                s_sb = [spool.tile([128, W], blur_dt, tag=f"s_{t}", name=f"s_{t}")
                        for t in range(NT)]
                for t in range(NT):
                    sig_in = du[t][:] if it == 0 else prev_dp[t][:]
                    nc.scalar.activation(s_sb[t][:], sig_in, AF.Sigmoid)

                A_iter = A_s if it < N_ITER - 1 else A_p
                # pass 1: UT[w, h'] chunks (transposing banded blur along h)
                ut = [spool.tile([128, H], blur_dt, tag=f"ut_{s}", name=f"ut_{s}")
                      for s in range(NT)]
                for s in range(NT):
                    UTP = psum()
                    band_mms(UTP,
                             lambda t, s=s: s_sb[t][:, 128 * s:128 * s + 128],
                             lambda t, a, b2: A_iter[:, t, a:b2])
                    nc.vector.tensor_copy(ut[s][:], UTP[:])

                # pass 2: V[h, w'] chunks back in row layout (+ injects)
                if it < N_ITER - 1:
                    new_dp = []
                    for tp in range(NT):
                        DP = psum()
                        band_mms(DP,
                                 lambda s, tp=tp: ut[s][:, 128 * tp:128 * tp + 128],
                                 lambda s, a, b2: A_iter[:, s, a:b2],
                                 last_extra=1)
                        # d' = 2*blur(s) - M
                        nc.tensor.matmul(
                            DP[:], negI[:], M[tp][:],
                            start=False, stop=True, skip_group_check=True)
                        new_dp.append(DP)
                    prev_dp = new_dp
                else:
                    # final: DP1 = blur(s) + u1 = out1 ; out0 = P2 - DP1
                    for tp in range(NT):
                        DP1 = psum()
                        band_mms(DP1,
                                 lambda s, tp=tp: ut[s][:, 128 * tp:128 * tp + 128],
                                 lambda s, a, b2: A_p[:, s, a:b2])
                        o1 = spool.tile([128, W], F32, tag=f"o1_{tp}", name=f"o1_{tp}")
                        o0 = spool.tile([128, W], F32, tag=f"o0_{tp}", name=f"o0_{tp}")
                        nc.vector.tensor_add(o1[:], DP1[:], u1[tp][:])
                        nc.vector.scalar_tensor_tensor(
                            o0[:], DP1[:], -1.0, P2[tp][:], ALU.mult, ALU.add)
                        ring2 = nc.sync if tp % 2 == 0 else nc.scalar
                        ring2.dma_start(y_d[im, 1, 128 * tp:128 * tp + 128, :], o1[:])
                        ring2.dma_start(y_d[im, 0, 128 * tp:128 * tp + 128, :], o0[:])


_CACHE = {}


def _get_compiled(conv_bf16=None, blur_bf16=None):
    conv_bf16 = CONV_BF16 if conv_bf16 is None else conv_bf16
    blur_bf16 = BLUR_BF16 if blur_bf16 is None else blur_bf16
    key = (conv_bf16, blur_bf16)
    if key in _CACHE:
        return _CACHE[key]
    conv_dt = BF16 if conv_bf16 else F32
    blur_dt = BF16 if blur_bf16 else F32
    nc = bacc.Bacc(
        "TRN2",
        target_bir_lowering=False,
        debug=False,
        enable_asserts=False,
        num_devices=N_CORES,
    )
    with tile.TileContext(nc) as tc:
        _build(nc, tc, conv_dt, blur_dt)
    nc.compile()
    _CACHE[key] = nc
    return nc


def host_constants(conv_w, conv_b, conv_bf16=None, blur_bf16=None):
    """All weight-derived device constants, as numpy arrays."""
    conv_bf16 = CONV_BF16 if conv_bf16 is None else conv_bf16
    blur_bf16 = BLUR_BF16 if blur_bf16 is None else blur_bf16
    w = np.asarray(conv_w, np.float32)
    b = np.asarray(conv_b, np.float32)
    sets = [w[1] + 0.0, w[1] - w[0]]  # u1-plane, du-plane (3,3,3) each

    bands = np.zeros((128, 18, 128), np.float32)
    r = np.arange(128)
    for set_i, ws in enumerate(sets):
        for c in range(3):
            for kx in range(3):
                Band = np.zeros((128, 128), np.float32)
                for ky in range(3):
                    m = r - (ky - 1)
                    ok = (m >= 0) & (m < 128)
                    Band[r[ok], m[ok]] = ws[c, ky, kx]
                bands[:, set_i * 9 + c * 3 + kx, :] = Band

    wf = np.zeros((35, 6, 128), np.float32)
    for set_i, ws in enumerate(sets):
        for kx in range(3):
            WF = np.zeros((35, 128), np.float32)
            for c in range(3):
                WF[0 + c, 0] = ws[c, 0, kx]      # r=0 rows: x row 128b-1, ky=0
                WF[32 + c, 127] = ws[c, 2, kx]   # r=1 rows: x row 128b+128, ky=2
            wf[:, set_i * 3 + kx, :] = WF

    def tile4(A):
        return np.ascontiguousarray(A.reshape(NT, 128, H).transpose(1, 0, 2))

    A_s = tile4(_make_A(np.sqrt(np.float32(2.0))))
    A_p = tile4(_make_A(1.0))

    k = _gauss_k()
    v = np.convolve(np.ones(H, np.float32), k, mode="same").astype(np.float32)
    ob_full = np.outer(v, v).astype(np.float32)  # blur(ones), rank-1
    ob = np.ascontiguousarray(ob_full.reshape(NT, 128, W).transpose(1, 0, 2))

    ident = np.eye(128, dtype=np.float32)
    cdt = ml_dtypes.bfloat16 if conv_bf16 else np.float32
    bdt = ml_dtypes.bfloat16 if blur_bf16 else np.float32
    b1, db = np.float32(b[1]), np.float32(b[1] - b[0])
    return {
        "bands": bands.astype(cdt),
        "wf": wf.astype(cdt),
        "A_s": A_s.astype(bdt),
        "A_p": A_p.astype(bdt),
        "negI": (-ident).astype(np.float16),
        "ob": ob,
        "biases": np.tile(np.array([[b1, db]], np.float32), (128, 1)),
    }


def _install_ntff_hook_shim():
    """This container's antenv lacks axon_hooks; recreate the NTFF profile
    hook via ctypes into libaxon_pjrt.so (same ABI trn_boot.py uses).
    Only invoked for traced (profiling) runs."""
    import types
    import ctypes
    import contextlib

    try:
        from antenv.axon_hooks import get_axon_ntff_profile_hook  # noqa: F401
        return
    except ImportError:
        pass

    hook = None
    so_path = "/opt/axon/libaxon_pjrt.so"
    if os.path.exists(so_path):
        lib = ctypes.CDLL(so_path)
        if hasattr(lib, "axon_start_nrt_profile"):
            lib.axon_start_nrt_profile.argtypes = [
                ctypes.POINTER(ctypes.c_int64), ctypes.c_size_t,
            ]
            lib.axon_start_nrt_profile.restype = ctypes.c_int64
            lib.axon_stop_nrt_profile.argtypes = [ctypes.c_char_p]
            lib.axon_stop_nrt_profile.restype = ctypes.c_int64

            @contextlib.contextmanager
            def _hook(output_dir, device_ids):
                import jax

                jax.devices()
                if device_ids:
                    ids = (ctypes.c_int64 * len(device_ids))(*device_ids)
                    rc = lib.axon_start_nrt_profile(ids, len(device_ids))
                else:
                    rc = lib.axon_start_nrt_profile(None, 0)
                if rc != 0:
                    raise RuntimeError(f"axon_start_nrt_profile rc={rc}")
                try:
                    yield
                finally:
                    n = lib.axon_stop_nrt_profile(str(output_dir).encode())
                    print(f"profile: {n} file(s) written to {output_dir}", file=sys.stderr)

            hook = _hook

    import antenv

    mod = types.ModuleType("antenv.axon_hooks")
    mod.get_axon_ntff_profile_hook = lambda: hook
    mod.set_axon_ntff_profile_hook = lambda h: None
    sys.modules["antenv.axon_hooks"] = mod
    antenv.axon_hooks = mod


def kernel(x, conv_w, conv_b, _trace=False, _return_results=False):
    if _trace:
        _install_ntff_hook_shim()
    x = np.ascontiguousarray(np.asarray(x, np.float32))
    consts = host_constants(conv_w, conv_b)

    nc = _get_compiled()
    in_maps = []
    for core in range(N_CORES):
        m = {"x": np.ascontiguousarray(x[IMGS_PER_CORE * core:IMGS_PER_CORE * (core + 1)])}
        m.update(consts)
        in_maps.append(m)

    res = run_bass_kernel_spmd(nc, in_maps, core_ids=list(range(N_CORES)), trace=_trace)
    out = np.concatenate([res.results[c]["y"] for c in range(N_CORES)], axis=0).astype(np.float32)
    if _return_results:
        return out, res
    return out


if __name__ == "__main__":
    rng = np.random.default_rng(0)
    x = rng.standard_normal((16, 3, H, W), dtype=np.float32)
    w = (rng.standard_normal((2, 3, 3, 3)) * 0.1).astype(np.float32)
    b = np.zeros(2, np.float32)
    y = kernel(x=x, conv_w=w, conv_b=b)
    print("out", y.shape, y.dtype)


# revision 22
# speedup vs baseline: 2.2320x; 1.1343x over previous
"""Trainium2 Bass kernel for CRFExtensionModule (conv3x3 backbone + 5 mean-field
CRF iterations with separable Gaussian blur).

Strategy (per NeuronCore, 2 images of the 16-image batch):
  - C=2 softmax collapses to a sigmoid of d = logit1 - logit0, and
    blur(q0) = blur(ones) - blur(q1), so the whole CRF loop is a single-plane
    recurrence:  d' = (du - ob) + 2*blur(sigmoid(d)),  ob = blur(ones).
  - conv3x3 computes only the planes u1 and du = u1 - u0 via banded matmuls on
    the TensorEngine (band stationary, x moving), with tiny K=6 "fix" matmuls
    for the 2 rows per 128-row tile boundary that the aligned K window misses.
  - blur = two *transposing* banded matmul passes on the TensorEngine:
    pass1: UT[w,h'] = sum_h s[h,w] A[h,h']   (lhsT = s block, rhs = A band)
    pass2: V[h,w'] = sum_w UT[w,h] A[w,w']   (lhsT = UT block, rhs = A band)
    Output lands back in [h, w] layout - no explicit transposes anywhere.
  - ob = blur(ones) = outer(v, v) is rank-1; M = ob - du is formed once per
    image and injected into PSUM with K=128 identity matmuls.
  - Final iteration: out1 = u1 + blur(s), out0 = (u1 + M) - blur(s) are both
    assembled fully in PSUM (plain/negated A + identity injects) and DMA'd
    directly PSUM -> DRAM.

kernel(**inputs) takes the FULL inputs and returns the FULL output.
"""

import os
import sys
from contextlib import ExitStack

sys.path.insert(0, "/opt/trn_rl_repo")

import numpy as np
import ml_dtypes

import concourse.bass as bass
import concourse.bacc as bacc
import concourse.tile as tile
import concourse.mybir as mybir
from concourse.bass_utils import run_bass_kernel_spmd

F32 = mybir.dt.float32
BF16 = mybir.dt.bfloat16
FP16 = mybir.dt.float16

# dtype knobs (bf16 halves TensorEngine streaming / weight-load time)
CONV_BF16 = os.environ.get("KERNEL_CONV_BF16", "1") == "1"
BLUR_BF16 = os.environ.get("KERNEL_BLUR_BF16", "1") == "1"

N_CORES = 8
IMGS_PER_CORE = 2
H = W = 512
NT = 4  # 128-row tiles per image plane
N_ITER = 5
FILT = 11


def _gauss_k():
    d = np.arange(FILT, dtype=np.float32) - np.float32((FILT - 1) / 2.0)
    k = np.exp(-(d ** 2) / np.float32(2.0)).astype(np.float32)
    return (k / k.sum()).astype(np.float32)


def _make_A(scale):
    """A[h, h'] = k[h-h'+5] for |h-h'| <= 5 (zero-padded 'SAME' 1D blur)."""
    k = (_gauss_k() * np.float32(scale)).astype(np.float32)
    A = np.zeros((H, H), np.float32)
    hp = np.arange(H)
    for j in range(FILT):
        h = hp + (j - 5)
        m = (h >= 0) & (h < H)
        A[h[m], hp[m]] = k[j]
    return A


def _win(t):
    """h' window that rows [128t, 128t+128) of A touch."""
    return max(0, 128 * t - 5), min(H, 128 * t + 133)


def _np_dt(dt):
    return ml_dtypes.bfloat16 if dt == BF16 else np.float32


# ---------------------------------------------------------------------------
# kernel body (traced once; shared SPMD program for all 8 cores)
# ---------------------------------------------------------------------------


def _build(nc, tc, conv_dt, blur_dt):
    x_d = nc.dram_tensor("x", [IMGS_PER_CORE, 3, H, W], F32, kind="ExternalInput").ap()
    y_d = nc.dram_tensor("y", [IMGS_PER_CORE, 2, H, W], F32, kind="ExternalOutput").ap()
    bands_d = nc.dram_tensor("bands", [128, 18, 128], conv_dt, kind="ExternalInput").ap()
    wf_d = nc.dram_tensor("wf", [35, 6, 128], conv_dt, kind="ExternalInput").ap()
    As_d = nc.dram_tensor("A_s", [128, NT, H], blur_dt, kind="ExternalInput").ap()
    Ap_d = nc.dram_tensor("A_p", [128, NT, H], blur_dt, kind="ExternalInput").ap()
    negI_d = nc.dram_tensor("negI", [128, 128], FP16, kind="ExternalInput").ap()
    ob_d = nc.dram_tensor("ob", [128, NT, W], F32, kind="ExternalInput").ap()
    biases_d = nc.dram_tensor("biases", [128, 2], F32, kind="ExternalInput").ap()

    ALU = mybir.AluOpType
    AF = mybir.ActivationFunctionType

    with ExitStack() as ctx:
        cpool = ctx.enter_context(tc.tile_pool(name="consts", bufs=1))
        spool = ctx.enter_context(tc.tile_pool(name="sbuf", bufs=2))
        xpool = ctx.enter_context(tc.tile_pool(name="xin", bufs=1))
        ppool = ctx.enter_context(
            tc.tile_pool(name="psum", bufs=8, space=bass.MemorySpace.PSUM))

        def psum():
            return ppool.tile([128, 512], F32, tag="ps", name="ps")

        def band_mms(P, lhsT_of, A_t, last_extra=0):
            """Banded blur matmuls into one PSUM bank; each window split into
            an accumulate-only overlap strip and a fresh strip (HW has
            per-element has_written, CoreSim requires purity)."""
            written = 0
            for t in range(NT):
                lo, hi = _win(t)
                if lo < written:
                    nc.tensor.matmul(
                        P[:, lo:written], lhsT_of(t), A_t(t, lo, written),
                        start=False, stop=False, skip_group_check=True)
                flo = max(lo, written)
                is_last = (t == NT - 1) and last_extra == 0
                nc.tensor.matmul(
                    P[:, flo:hi], lhsT_of(t), A_t(t, flo, hi),
                    start=(t == 0), stop=is_last, skip_group_check=True)
                written = hi

        # --- constants into SBUF (once; all dtype-matched HWDGE loads) ---
        bands = cpool.tile([128, 18, 128], conv_dt, tag="bands")
        nc.scalar.dma_start(bands[:], bands_d)
        wf = cpool.tile([35, 6, 128], conv_dt, tag="wf")
        nc.scalar.dma_start(wf[:], wf_d)
        A_s = cpool.tile([128, NT, H], blur_dt, tag="A_s")
        A_p = cpool.tile([128, NT, H], blur_dt, tag="A_p")
        nc.scalar.dma_start(A_s[:], As_d)
        nc.scalar.dma_start(A_p[:], Ap_d)
        negI = cpool.tile([128, 128], FP16, tag="negI")
        nc.scalar.dma_start(negI[:], negI_d)
        ob = cpool.tile([128, NT, W], F32, tag="ob")
        nc.scalar.dma_start(ob[:], ob_d)
        biases = cpool.tile([128, 2], F32, tag="biases")
        nc.scalar.dma_start(biases[:], biases_d)

        for im in range(IMGS_PER_CORE):
            # ---- boundary-fix rows first (SWDGE cast ring warms up early):
            #   xb[b][3r + c, j]; r=0 (parts 0-2) -> x row 128b-1, r=1 -> 128b+128
            xbt = []
            for b in range(NT):
                # r=0 rows at partitions 0-2, r=1 rows at 32-34 (SWDGE dst
                # partition starts must be 32-aligned)
                xb = xpool.tile([35, W], conv_dt, tag=f"xb{im}{b}", name=f"xb{im}{b}")
                nc.vector.memset(xb[:], 0.0)
                if b > 0:
                    nc.gpsimd.dma_start(xb[0:3, :], x_d[im, :, 128 * b - 1, :])
                if b < NT - 1:
                    nc.gpsimd.dma_start(xb[32:35, :], x_d[im, :, 128 * b + 128, :])
                xbt.append(xb)

            # ---- x planes: per-channel SWDGE cast-DMAs (conv starts as
            #      soon as channel 0 lands; no staging, no Q7 cast work) ----
            xt = xpool.tile([128, 3, NT, W], conv_dt, tag=f"xt{im}", name=f"xt{im}")
            if im == 0:
                for c in range(3):
                    nc.gpsimd.dma_start(
                        xt[:, c, :, :],
                        x_d[im, c].rearrange("(b p) w -> p b w", p=128))
            else:
                nc.gpsimd.dma_start(
                    xt[:], x_d[im].rearrange("c (b p) w -> p c b w", p=128))

            # ---- conv: u1 / du planes (+ bias), one PSUM bank per (set, b) ----
            u1 = [spool.tile([128, W], F32, tag=f"u1_{b}", name=f"u1_{b}") for b in range(NT)]
            du = [spool.tile([128, W], F32, tag=f"du_{b}", name=f"du_{b}") for b in range(NT)]
            M = [spool.tile([128, W], FP16, tag=f"M_{b}", name=f"M_{b}") for b in range(NT)]
            P2 = [spool.tile([128, W], F32, tag=f"P2_{b}", name=f"P2_{b}") for b in range(NT)]
            for b in range(NT):
                for set_i in range(2):
                    P = psum()
                    n_mm = 0
                    for c in range(3):
                        for kx in (1, 0, 2):  # center first: full-bank start=True
                            # kx=0 reads x[.., j-1]: src cols [0,511) -> out [1,512)
                            # kx=2 reads x[.., j+1]: src cols [1,512) -> out [0,511)
                            sl, ol = (0, 1) if kx == 0 else (1, 0) if kx == 2 else (0, 0)
                            n = W - (1 if kx != 1 else 0)
                            nc.tensor.matmul(
                                P[:, ol:ol + n],
                                bands[:, set_i * 9 + c * 3 + kx, :],
                                xt[:, c, b, sl:sl + n],
                                start=(n_mm == 0), stop=False,
                                skip_group_check=True)
                            n_mm += 1
                    for kx in (1, 0, 2):
                        sl, ol = (0, 1) if kx == 0 else (1, 0) if kx == 2 else (0, 0)
                        n = W - (1 if kx != 1 else 0)
                        nc.tensor.matmul(
                            P[:, ol:ol + n], wf[:, set_i * 3 + kx, :],
                            xbt[b][:, sl:sl + n],
                            start=False, stop=(kx == 2), skip_group_check=True)
                    if set_i == 0:
                        # u1 = conv1 + b1   (ACT)
                        nc.scalar.activation(
                            u1[b][:], P[:], AF.Identity,
                            bias=biases[:, 0:1], scale=1.0)
                    else:
                        # du = (conv1-conv0) + (b1-b0)   (DVE)
                        nc.vector.tensor_scalar(
                            du[b][:], P[:], biases[:, 1:2], None, ALU.add)
                # M = ob - du (fp16: inject operand) ; P2 = u1 + M
                nc.vector.tensor_sub(M[b][:], ob[:, b, :], du[b][:])
                nc.vector.tensor_add(P2[b][:], u1[b][:], M[b][:])

            # ---- CRF iterations ----
            prev_dp = None  # 4 single-bank psum tiles holding d' chunks
            for it in range(N_ITER):
                s_sb = [spool.tile([128, W], blur_dt, tag=f"s_{t}", name=f"s_{t}")
                        for t in range(NT)]
                for t in range(NT):
                    sig_in = du[t][:] if it == 0 else prev_dp[t][:]
                    nc.scalar.activation(s_sb[t][:], sig_in, AF.Sigmoid)

                A_iter = A_s if it < N_ITER - 1 else A_p
                # pass 1: UT[w, h'] chunks (transposing banded blur along h)
                ut = [spool.tile([128, H], blur_dt, tag=f"ut_{s}", name=f"ut_{s}")
                      for s in range(NT)]
                for s in range(NT):
                    UTP = psum()
                    band_mms(UTP,
                             lambda t, s=s: s_sb[t][:, 128 * s:128 * s + 128],
                             lambda t, a, b2: A_iter[:, t, a:b2])
                    nc.vector.tensor_copy(ut[s][:], UTP[:])

                # pass 2: V[h, w'] chunks back in row layout (+ injects)
                if it < N_ITER - 1:
                    new_dp = []
                    for tp in range(NT):
                        DP = psum()
                        band_mms(DP,
                                 lambda s, tp=tp: ut[s][:, 128 * tp:128 * tp + 128],
                                 lambda s, a, b2: A_iter[:, s, a:b2],
                                 last_extra=1)
                        # d' = 2*blur(s) - M
                        nc.tensor.matmul(
                            DP[:], negI[:], M[tp][:],
                            start=False, stop=True, skip_group_check=True)
                        new_dp.append(DP)
                    prev_dp = new_dp
                else:
                    # final: DP1 = blur(s) + u1 = out1 ; out0 = P2 - DP1
                    for tp in range(NT):
                        DP1 = psum()
                        band_mms(DP1,
                                 lambda s, tp=tp: ut[s][:, 128 * tp:128 * tp + 128],
                                 lambda s, a, b2: A_p[:, s, a:b2])
                        o1 = spool.tile([128, W], F32, tag=f"o1_{tp}", name=f"o1_{tp}")
                        o0 = spool.tile([128, W], F32, tag=f"o0_{tp}", name=f"o0_{tp}")
                        nc.vector.tensor_add(o1[:], DP1[:], u1[tp][:])
                        nc.vector.scalar_tensor_tensor(
                            o0[:], DP1[:], -1.0, P2[tp][:], ALU.mult, ALU.add)
                        ring2 = nc.sync if tp % 2 == 0 else nc.scalar
                        ring2.dma_start(y_d[im, 1, 128 * tp:128 * tp + 128, :], o1[:])
                        ring2.dma_start(y_d[im, 0, 128 * tp:128 * tp + 128, :], o0[:])


_CACHE = {}


def _get_compiled(conv_bf16=None, blur_bf16=None):
    conv_bf16 = CONV_BF16 if conv_bf16 is None else conv_bf16
    blur_bf16 = BLUR_BF16 if blur_bf16 is None else blur_bf16
    key = (conv_bf16, blur_bf16)
    if key in _CACHE:
        return _CACHE[key]
    conv_dt = BF16 if conv_bf16 else F32
    blur_dt = BF16 if blur_bf16 else F32
    nc = bacc.Bacc(
        "TRN2",
        target_bir_lowering=False,
        debug=False,
        enable_asserts=False,
        num_devices=N_CORES,
    )
    with tile.TileContext(nc) as tc:
        _build(nc, tc, conv_dt, blur_dt)
    nc.compile()
    _CACHE[key] = nc
    return nc


def host_constants(conv_w, conv_b, conv_bf16=None, blur_bf16=None):
    """All weight-derived device constants, as numpy arrays."""
    conv_bf16 = CONV_BF16 if conv_bf16 is None else conv_bf16
    blur_bf16 = BLUR_BF16 if blur_bf16 is None else blur_bf16
    w = np.asarray(conv_w, np.float32)
    b = np.asarray(conv_b, np.float32)
    sets = [w[1] + 0.0, w[1] - w[0]]  # u1-plane, du-plane (3,3,3) each

    bands = np.zeros((128, 18, 128), np.float32)
    r = np.arange(128)
    for set_i, ws in enumerate(sets):
        for c in range(3):
            for kx in range(3):
                Band = np.zeros((128, 128), np.float32)
                for ky in range(3):
                    m = r - (ky - 1)
                    ok = (m >= 0) & (m < 128)
                    Band[r[ok], m[ok]] = ws[c, ky, kx]
                bands[:, set_i * 9 + c * 3 + kx, :] = Band

    wf = np.zeros((35, 6, 128), np.float32)
    for set_i, ws in enumerate(sets):
        for kx in range(3):
            WF = np.zeros((35, 128), np.float32)
            for c in range(3):
                WF[0 + c, 0] = ws[c, 0, kx]      # r=0 rows: x row 128b-1, ky=0
                WF[32 + c, 127] = ws[c, 2, kx]   # r=1 rows: x row 128b+128, ky=2
            wf[:, set_i * 3 + kx, :] = WF

    def tile4(A):
        return np.ascontiguousarray(A.reshape(NT, 128, H).transpose(1, 0, 2))

    A_s = tile4(_make_A(np.sqrt(np.float32(2.0))))
    A_p = tile4(_make_A(1.0))

    k = _gauss_k()
    v = np.convolve(np.ones(H, np.float32), k, mode="same").astype(np.float32)
    ob_full = np.outer(v, v).astype(np.float32)  # blur(ones), rank-1
    ob = np.ascontiguousarray(ob_full.reshape(NT, 128, W).transpose(1, 0, 2))

    ident = np.eye(128, dtype=np.float32)
    cdt = ml_dtypes.bfloat16 if conv_bf16 else np.float32
    bdt = ml_dtypes.bfloat16 if blur_bf16 else np.float32
    b1, db = np.float32(b[1]), np.float32(b[1] - b[0])
    return {
        "bands": bands.astype(cdt),
        "wf": wf.astype(cdt),
        "A_s": A_s.astype(bdt),
        "A_p": A_p.astype(bdt),
        "negI": (-ident).astype(np.float16),
        "ob": ob,
        "biases": np.tile(np.array([[b1, db]], np.float32), (128, 1)),
    }


def _install_ntff_hook_shim():
    """This container's antenv lacks axon_hooks; recreate the NTFF profile
    hook via ctypes into libaxon_pjrt.so (same ABI trn_boot.py uses).
    Only invoked for traced (profiling) runs."""
    import types
    import ctypes
    import contextlib

    try:
        from antenv.axon_hooks import get_axon_ntff_profile_hook  # noqa: F401
        return
    except ImportError:
        pass

    hook = None
    so_path = "/opt/axon/libaxon_pjrt.so"
    if os.path.exists(so_path):
        lib = ctypes.CDLL(so_path)
        if hasattr(lib, "axon_start_nrt_profile"):
            lib.axon_start_nrt_profile.argtypes = [
                ctypes.POINTER(ctypes.c_int64), ctypes.c_size_t,
            ]
            lib.axon_start_nrt_profile.restype = ctypes.c_int64
            lib.axon_stop_nrt_profile.argtypes = [ctypes.c_char_p]
            lib.axon_stop_nrt_profile.restype = ctypes.c_int64

            @contextlib.contextmanager
            def _hook(output_dir, device_ids):
                import jax

                jax.devices()
                if device_ids:
                    ids = (ctypes.c_int64 * len(device_ids))(*device_ids)
                    rc = lib.axon_start_nrt_profile(ids, len(device_ids))
                else:
                    rc = lib.axon_start_nrt_profile(None, 0)
                if rc != 0:
                    raise RuntimeError(f"axon_start_nrt_profile rc={rc}")
                try:
                    yield
                finally:
                    n = lib.axon_stop_nrt_profile(str(output_dir).encode())
                    print(f"profile: {n} file(s) written to {output_dir}", file=sys.stderr)

            hook = _hook

    import antenv

    mod = types.ModuleType("antenv.axon_hooks")
    mod.get_axon_ntff_profile_hook = lambda: hook
    mod.set_axon_ntff_profile_hook = lambda h: None
    sys.modules["antenv.axon_hooks"] = mod
    antenv.axon_hooks = mod


def kernel(x, conv_w, conv_b, _trace=False, _return_results=False):
    if _trace:
        _install_ntff_hook_shim()
    x = np.ascontiguousarray(np.asarray(x, np.float32))
    consts = host_constants(conv_w, conv_b)

    nc = _get_compiled()
    in_maps = []
    for core in range(N_CORES):
        m = {"x": np.ascontiguousarray(x[IMGS_PER_CORE * core:IMGS_PER_CORE * (core + 1)])}
        m.update(consts)
        in_maps.append(m)

    res = run_bass_kernel_spmd(nc, in_maps, core_ids=list(range(N_CORES)), trace=_trace)
    out = np.concatenate([res.results[c]["y"] for c in range(N_CORES)], axis=0).astype(np.float32)
    if _return_results:
        return out, res
    return out


if __name__ == "__main__":
    rng = np.random.default_rng(0)
    x = rng.standard_normal((16, 3, H, W), dtype=np.float32)
    w = (rng.standard_normal((2, 3, 3, 3)) * 0.1).astype(np.float32)
    b = np.zeros(2, np.float32)
    y = kernel(x=x, conv_w=w, conv_b=b)
    print("out", y.shape, y.dtype)


# revision 23
# speedup vs baseline: 2.3404x; 1.0486x over previous
"""Trainium2 Bass kernel for CRFExtensionModule (conv3x3 backbone + 5 mean-field
CRF iterations with separable Gaussian blur).

Strategy (per NeuronCore, 2 images of the 16-image batch):
  - C=2 softmax collapses to a sigmoid of d = logit1 - logit0, and
    blur(q0) = blur(ones) - blur(q1), so the whole CRF loop is a single-plane
    recurrence:  d' = (du - ob) + 2*blur(sigmoid(d)),  ob = blur(ones).
  - conv3x3 computes only the planes u1 and du = u1 - u0 via banded matmuls on
    the TensorEngine (band stationary, x moving), with tiny K=6 "fix" matmuls
    for the 2 rows per 128-row tile boundary that the aligned K window misses.
  - blur = two *transposing* banded matmul passes on the TensorEngine:
    pass1: UT[w,h'] = sum_h s[h,w] A[h,h']   (lhsT = s block, rhs = A band)
    pass2: V[h,w'] = sum_w UT[w,h] A[w,w']   (lhsT = UT block, rhs = A band)
    Output lands back in [h, w] layout - no explicit transposes anywhere.
  - ob = blur(ones) = outer(v, v) is rank-1; M = ob - du is formed once per
    image and injected into PSUM with K=128 identity matmuls.
  - Final iteration: out1 = u1 + blur(s), out0 = (u1 + M) - blur(s) are both
    assembled fully in PSUM (plain/negated A + identity injects) and DMA'd
    directly PSUM -> DRAM.

kernel(**inputs) takes the FULL inputs and returns the FULL output.
"""

import os
import sys
from contextlib import ExitStack

sys.path.insert(0, "/opt/trn_rl_repo")

import numpy as np
import ml_dtypes

import concourse.bass as bass
import concourse.bacc as bacc
import concourse.tile as tile
import concourse.mybir as mybir
from concourse.bass_utils import run_bass_kernel_spmd

F32 = mybir.dt.float32
BF16 = mybir.dt.bfloat16
FP16 = mybir.dt.float16

# dtype knobs (bf16 halves TensorEngine streaming / weight-load time)
CONV_BF16 = os.environ.get("KERNEL_CONV_BF16", "1") == "1"
BLUR_BF16 = os.environ.get("KERNEL_BLUR_BF16", "1") == "1"

N_CORES = 8
IMGS_PER_CORE = 2
H = W = 512
NT = 4  # 128-row tiles per image plane
N_ITER = 5
FILT = 11


def _gauss_k():
    d = np.arange(FILT, dtype=np.float32) - np.float32((FILT - 1) / 2.0)
    k = np.exp(-(d ** 2) / np.float32(2.0)).astype(np.float32)
    return (k / k.sum()).astype(np.float32)


def _make_A(scale):
    """A[h, h'] = k[h-h'+5] for |h-h'| <= 5 (zero-padded 'SAME' 1D blur)."""
    k = (_gauss_k() * np.float32(scale)).astype(np.float32)
    A = np.zeros((H, H), np.float32)
    hp = np.arange(H)
    for j in range(FILT):
        h = hp + (j - 5)
        m = (h >= 0) & (h < H)
        A[h[m], hp[m]] = k[j]
    return A


def _win(t):
    """h' window that rows [128t, 128t+128) of A touch."""
    return max(0, 128 * t - 5), min(H, 128 * t + 133)


def _np_dt(dt):
    return ml_dtypes.bfloat16 if dt == BF16 else np.float32


# ---------------------------------------------------------------------------
# kernel body (traced once; shared SPMD program for all 8 cores)
# ---------------------------------------------------------------------------


def _build(nc, tc, conv_dt, blur_dt):
    x_d = nc.dram_tensor("x", [IMGS_PER_CORE, 3, H, W], F32, kind="ExternalInput").ap()
    y_d = nc.dram_tensor("y", [IMGS_PER_CORE, 2, H, W], F32, kind="ExternalOutput").ap()
    bands_d = nc.dram_tensor("bands", [128, 18, 128], conv_dt, kind="ExternalInput").ap()
    wf_d = nc.dram_tensor("wf", [35, 6, 128], conv_dt, kind="ExternalInput").ap()
    As_d = nc.dram_tensor("A_s", [128, NT, H], blur_dt, kind="ExternalInput").ap()
    Ap_d = nc.dram_tensor("A_p", [128, NT, H], blur_dt, kind="ExternalInput").ap()
    negI_d = nc.dram_tensor("negI", [128, 128], FP16, kind="ExternalInput").ap()
    ob_d = nc.dram_tensor("ob", [128, NT, W], F32, kind="ExternalInput").ap()
    biases_d = nc.dram_tensor("biases", [128, 2], F32, kind="ExternalInput").ap()

    ALU = mybir.AluOpType
    AF = mybir.ActivationFunctionType

    with ExitStack() as ctx:
        cpool = ctx.enter_context(tc.tile_pool(name="consts", bufs=1))
        spool = ctx.enter_context(tc.tile_pool(name="sbuf", bufs=2))
        xpool = ctx.enter_context(tc.tile_pool(name="xin", bufs=1))
        ppool = ctx.enter_context(
            tc.tile_pool(name="psum", bufs=8, space=bass.MemorySpace.PSUM))

        def psum():
            return ppool.tile([128, 512], F32, tag="ps", name="ps")

        def band_mms(P, lhsT_of, A_t, last_extra=0):
            """Banded blur matmuls into one PSUM bank; each window split into
            an accumulate-only overlap strip and a fresh strip (HW has
            per-element has_written, CoreSim requires purity)."""
            written = 0
            for t in range(NT):
                lo, hi = _win(t)
                if lo < written:
                    nc.tensor.matmul(
                        P[:, lo:written], lhsT_of(t), A_t(t, lo, written),
                        start=False, stop=False, skip_group_check=True)
                flo = max(lo, written)
                is_last = (t == NT - 1) and last_extra == 0
                nc.tensor.matmul(
                    P[:, flo:hi], lhsT_of(t), A_t(t, flo, hi),
                    start=(t == 0), stop=is_last, skip_group_check=True)
                written = hi

        # --- constants into SBUF (once; all dtype-matched HWDGE loads) ---
        bands = cpool.tile([128, 18, 128], conv_dt, tag="bands")
        nc.scalar.dma_start(bands[:], bands_d)
        wf = cpool.tile([35, 6, 128], conv_dt, tag="wf")
        nc.scalar.dma_start(wf[:], wf_d)
        A_s = cpool.tile([128, NT, H], blur_dt, tag="A_s")
        A_p = cpool.tile([128, NT, H], blur_dt, tag="A_p")
        nc.scalar.dma_start(A_s[:], As_d)
        nc.scalar.dma_start(A_p[:], Ap_d)
        negI = cpool.tile([128, 128], FP16, tag="negI")
        nc.scalar.dma_start(negI[:], negI_d)
        ob = cpool.tile([128, NT, W], F32, tag="ob")
        nc.scalar.dma_start(ob[:], ob_d)
        biases = cpool.tile([128, 2], F32, tag="biases")
        nc.scalar.dma_start(biases[:], biases_d)

        for im in range(IMGS_PER_CORE):
            # ---- x planes: per-channel SWDGE cast-DMAs (conv starts as
            #      soon as channel 0 lands; no staging, no Q7 cast work) ----
            xt = xpool.tile([128, 3, NT, W], conv_dt, tag=f"xt{im}", name=f"xt{im}")
            if im == 0:
                for c in range(3):
                    nc.gpsimd.dma_start(
                        xt[:, c, :, :],
                        x_d[im, c].rearrange("(b p) w -> p b w", p=128))
            else:
                nc.gpsimd.dma_start(
                    xt[:], x_d[im].rearrange("c (b p) w -> p c b w", p=128))

            # ---- boundary-fix rows first (SWDGE cast ring warms up early):
            #   xb[b][3r + c, j]; r=0 (parts 0-2) -> x row 128b-1, r=1 -> 128b+128
            xbt = []
            for b in range(NT):
                # r=0 rows at partitions 0-2, r=1 rows at 32-34 (SWDGE dst
                # partition starts must be 32-aligned)
                xb = xpool.tile([35, W], conv_dt, tag=f"xb{im}{b}", name=f"xb{im}{b}")
                nc.vector.memset(xb[:], 0.0)
                if b > 0:
                    nc.gpsimd.dma_start(xb[0:3, :], x_d[im, :, 128 * b - 1, :])
                if b < NT - 1:
                    nc.gpsimd.dma_start(xb[32:35, :], x_d[im, :, 128 * b + 128, :])
                xbt.append(xb)

            # ---- conv: u1 / du planes (+ bias), one PSUM bank per (set, b) ----
            u1 = [spool.tile([128, W], F32, tag=f"u1_{b}", name=f"u1_{b}") for b in range(NT)]
            du = [spool.tile([128, W], F32, tag=f"du_{b}", name=f"du_{b}") for b in range(NT)]
            M = [spool.tile([128, W], FP16, tag=f"M_{b}", name=f"M_{b}") for b in range(NT)]
            P2 = [spool.tile([128, W], F32, tag=f"P2_{b}", name=f"P2_{b}") for b in range(NT)]
            for b in range(NT):
                for set_i in range(2):
                    P = psum()
                    n_mm = 0
                    for c in range(3):
                        for kx in (1, 0, 2):  # center first: full-bank start=True
                            # kx=0 reads x[.., j-1]: src cols [0,511) -> out [1,512)
                            # kx=2 reads x[.., j+1]: src cols [1,512) -> out [0,511)
                            sl, ol = (0, 1) if kx == 0 else (1, 0) if kx == 2 else (0, 0)
                            n = W - (1 if kx != 1 else 0)
                            nc.tensor.matmul(
                                P[:, ol:ol + n],
                                bands[:, set_i * 9 + c * 3 + kx, :],
                                xt[:, c, b, sl:sl + n],
                                start=(n_mm == 0), stop=False,
                                skip_group_check=True)
                            n_mm += 1
                    for kx in (1, 0, 2):
                        sl, ol = (0, 1) if kx == 0 else (1, 0) if kx == 2 else (0, 0)
                        n = W - (1 if kx != 1 else 0)
                        nc.tensor.matmul(
                            P[:, ol:ol + n], wf[:, set_i * 3 + kx, :],
                            xbt[b][:, sl:sl + n],
                            start=False, stop=(kx == 2), skip_group_check=True)
                    if set_i == 0:
                        # u1 = conv1 + b1   (ACT)
                        nc.scalar.activation(
                            u1[b][:], P[:], AF.Identity,
                            bias=biases[:, 0:1], scale=1.0)
                    else:
                        # du = (conv1-conv0) + (b1-b0)   (DVE)
                        nc.vector.tensor_scalar(
                            du[b][:], P[:], biases[:, 1:2], None, ALU.add)
                # M = ob - du (fp16: inject operand) ; P2 = u1 + M
                nc.gpsimd.tensor_sub(M[b][:], ob[:, b, :], du[b][:])
                nc.gpsimd.tensor_add(P2[b][:], u1[b][:], M[b][:])

            # ---- CRF iterations ----
            prev_dp = None  # 4 single-bank psum tiles holding d' chunks
            for it in range(N_ITER):
                s_sb = [spool.tile([128, W], blur_dt, tag=f"s_{t}", name=f"s_{t}")
                        for t in range(NT)]
                for t in range(NT):
                    sig_in = du[t][:] if it == 0 else prev_dp[t][:]
                    nc.scalar.activation(s_sb[t][:], sig_in, AF.Sigmoid)

                A_iter = A_s if it < N_ITER - 1 else A_p
                # pass 1: UT[w, h'] chunks (transposing banded blur along h)
                ut = [spool.tile([128, H], blur_dt, tag=f"ut_{s}", name=f"ut_{s}")
                      for s in range(NT)]
                for s in range(NT):
                    UTP = psum()
                    band_mms(UTP,
                             lambda t, s=s: s_sb[t][:, 128 * s:128 * s + 128],
                             lambda t, a, b2: A_iter[:, t, a:b2])
                    if s % 2 == 0:
                        nc.vector.tensor_copy(ut[s][:], UTP[:])
                    else:
                        nc.scalar.copy(ut[s][:], UTP[:])

                # pass 2: V[h, w'] chunks back in row layout (+ injects)
                if it < N_ITER - 1:
                    new_dp = []
                    for tp in range(NT):
                        DP = psum()
                        band_mms(DP,
                                 lambda s, tp=tp: ut[s][:, 128 * tp:128 * tp + 128],
                                 lambda s, a, b2: A_iter[:, s, a:b2],
                                 last_extra=1)
                        # d' = 2*blur(s) - M
                        nc.tensor.matmul(
                            DP[:], negI[:], M[tp][:],
                            start=False, stop=True, skip_group_check=True)
                        new_dp.append(DP)
                    prev_dp = new_dp
                else:
                    # final: DP1 = blur(s) + u1 = out1 ; out0 = P2 - DP1
                    for tp in range(NT):
                        DP1 = psum()
                        band_mms(DP1,
                                 lambda s, tp=tp: ut[s][:, 128 * tp:128 * tp + 128],
                                 lambda s, a, b2: A_p[:, s, a:b2])
                        o1 = spool.tile([128, W], F32, tag=f"o1_{tp}", name=f"o1_{tp}")
                        o0 = spool.tile([128, W], F32, tag=f"o0_{tp}", name=f"o0_{tp}")
                        nc.vector.tensor_add(o1[:], DP1[:], u1[tp][:])
                        nc.vector.scalar_tensor_tensor(
                            o0[:], DP1[:], -1.0, P2[tp][:], ALU.mult, ALU.add)
                        ring2 = nc.sync if tp % 2 == 0 else nc.scalar
                        ring2.dma_start(y_d[im, 1, 128 * tp:128 * tp + 128, :], o1[:])
                        ring2.dma_start(y_d[im, 0, 128 * tp:128 * tp + 128, :], o0[:])


_CACHE = {}


def _get_compiled(conv_bf16=None, blur_bf16=None):
    conv_bf16 = CONV_BF16 if conv_bf16 is None else conv_bf16
    blur_bf16 = BLUR_BF16 if blur_bf16 is None else blur_bf16
    key = (conv_bf16, blur_bf16)
    if key in _CACHE:
        return _CACHE[key]
    conv_dt = BF16 if conv_bf16 else F32
    blur_dt = BF16 if blur_bf16 else F32
    nc = bacc.Bacc(
        "TRN2",
        target_bir_lowering=False,
        debug=False,
        enable_asserts=False,
        num_devices=N_CORES,
    )
    with tile.TileContext(nc) as tc:
        _build(nc, tc, conv_dt, blur_dt)
    nc.compile()
    _CACHE[key] = nc
    return nc


def host_constants(conv_w, conv_b, conv_bf16=None, blur_bf16=None):
    """All weight-derived device constants, as numpy arrays."""
    conv_bf16 = CONV_BF16 if conv_bf16 is None else conv_bf16
    blur_bf16 = BLUR_BF16 if blur_bf16 is None else blur_bf16
    w = np.asarray(conv_w, np.float32)
    b = np.asarray(conv_b, np.float32)
    sets = [w[1] + 0.0, w[1] - w[0]]  # u1-plane, du-plane (3,3,3) each

    bands = np.zeros((128, 18, 128), np.float32)
    r = np.arange(128)
    for set_i, ws in enumerate(sets):
        for c in range(3):
            for kx in range(3):
                Band = np.zeros((128, 128), np.float32)
                for ky in range(3):
                    m = r - (ky - 1)
                    ok = (m >= 0) & (m < 128)
                    Band[r[ok], m[ok]] = ws[c, ky, kx]
                bands[:, set_i * 9 + c * 3 + kx, :] = Band

    wf = np.zeros((35, 6, 128), np.float32)
    for set_i, ws in enumerate(sets):
        for kx in range(3):
            WF = np.zeros((35, 128), np.float32)
            for c in range(3):
                WF[0 + c, 0] = ws[c, 0, kx]      # r=0 rows: x row 128b-1, ky=0
                WF[32 + c, 127] = ws[c, 2, kx]   # r=1 rows: x row 128b+128, ky=2
            wf[:, set_i * 3 + kx, :] = WF

    def tile4(A):
        return np.ascontiguousarray(A.reshape(NT, 128, H).transpose(1, 0, 2))

    A_s = tile4(_make_A(np.sqrt(np.float32(2.0))))
    A_p = tile4(_make_A(1.0))

    k = _gauss_k()
    v = np.convolve(np.ones(H, np.float32), k, mode="same").astype(np.float32)
    ob_full = np.outer(v, v).astype(np.float32)  # blur(ones), rank-1
    ob = np.ascontiguousarray(ob_full.reshape(NT, 128, W).transpose(1, 0, 2))

    ident = np.eye(128, dtype=np.float32)
    cdt = ml_dtypes.bfloat16 if conv_bf16 else np.float32
    bdt = ml_dtypes.bfloat16 if blur_bf16 else np.float32
    b1, db = np.float32(b[1]), np.float32(b[1] - b[0])
    return {
        "bands": bands.astype(cdt),
        "wf": wf.astype(cdt),
        "A_s": A_s.astype(bdt),
        "A_p": A_p.astype(bdt),
        "negI": (-ident).astype(np.float16),
        "ob": ob,
        "biases": np.tile(np.array([[b1, db]], np.float32), (128, 1)),
    }


def _install_ntff_hook_shim():
    """This container's antenv lacks axon_hooks; recreate the NTFF profile
    hook via ctypes into libaxon_pjrt.so (same ABI trn_boot.py uses).
    Only invoked for traced (profiling) runs."""
    import types
    import ctypes
    import contextlib

    try:
        from antenv.axon_hooks import get_axon_ntff_profile_hook  # noqa: F401
        return
    except ImportError:
        pass

    hook = None
    so_path = "/opt/axon/libaxon_pjrt.so"
    if os.path.exists(so_path):
        lib = ctypes.CDLL(so_path)
        if hasattr(lib, "axon_start_nrt_profile"):
            lib.axon_start_nrt_profile.argtypes = [
                ctypes.POINTER(ctypes.c_int64), ctypes.c_size_t,
            ]
            lib.axon_start_nrt_profile.restype = ctypes.c_int64
            lib.axon_stop_nrt_profile.argtypes = [ctypes.c_char_p]
            lib.axon_stop_nrt_profile.restype = ctypes.c_int64

            @contextlib.contextmanager
            def _hook(output_dir, device_ids):
                import jax

                jax.devices()
                if device_ids:
                    ids = (ctypes.c_int64 * len(device_ids))(*device_ids)
                    rc = lib.axon_start_nrt_profile(ids, len(device_ids))
                else:
                    rc = lib.axon_start_nrt_profile(None, 0)
                if rc != 0:
                    raise RuntimeError(f"axon_start_nrt_profile rc={rc}")
                try:
                    yield
                finally:
                    n = lib.axon_stop_nrt_profile(str(output_dir).encode())
                    print(f"profile: {n} file(s) written to {output_dir}", file=sys.stderr)

            hook = _hook

    import antenv

    mod = types.ModuleType("antenv.axon_hooks")
    mod.get_axon_ntff_profile_hook = lambda: hook
    mod.set_axon_ntff_profile_hook = lambda h: None
    sys.modules["antenv.axon_hooks"] = mod
    antenv.axon_hooks = mod


def kernel(x, conv_w, conv_b, _trace=False, _return_results=False):
    if _trace:
        _install_ntff_hook_shim()
    x = np.ascontiguousarray(np.asarray(x, np.float32))
    consts = host_constants(conv_w, conv_b)

    nc = _get_compiled()
    in_maps = []
    for core in range(N_CORES):
        m = {"x": np.ascontiguousarray(x[IMGS_PER_CORE * core:IMGS_PER_CORE * (core + 1)])}
        m.update(consts)
        in_maps.append(m)

    res = run_bass_kernel_spmd(nc, in_maps, core_ids=list(range(N_CORES)), trace=_trace)
    out = np.concatenate([res.results[c]["y"] for c in range(N_CORES)], axis=0).astype(np.float32)
    if _return_results:
        return out, res
    return out


if __name__ == "__main__":
    rng = np.random.default_rng(0)
    x = rng.standard_normal((16, 3, H, W), dtype=np.float32)
    w = (rng.standard_normal((2, 3, 3, 3)) * 0.1).astype(np.float32)
    b = np.zeros(2, np.float32)
    y = kernel(x=x, conv_w=w, conv_b=b)
    print("out", y.shape, y.dtype)
